# revision 1
# baseline (speedup 1.0000x reference)
import numpy as np
import jax
import jax.numpy as jnp
from jax import lax

# Problem constants (hardcoded per spec: nn_AxialAttentionWithPosition3D)
G = 8        # groups
GP = 8       # group planes
K = 56       # attention axis length
OP = 64      # out planes
EPS = 1e-5
NCORES = 8
D1 = 32      # seq axis, sharded 4 per core
D2 = 32
C_IN = 64
B_LOC = (D1 // NCORES) * D2   # 128 positions per core
N_BN1 = NCORES * B_LOC * K    # global BN1/BN3 sample count per channel
N_BN2 = NCORES * B_LOC * K * K

jax.config.update("jax_default_matmul_precision", "highest")


def _shard_fn(xs, w_qkv, bn_qkv_g, bn_qkv_b, bn_sim_g, bn_sim_b,
              bn_out_g, bn_out_b, q_emb, k_emb, v_emb):
    # xs: [1, 64, D1/8, K, D2] slab of x along D1
    xp = jnp.transpose(xs, (0, 2, 4, 1, 3))          # [1, d1l, D2, C, K]
    xb = xp.reshape(B_LOC, C_IN, K)

    qkv = jnp.einsum('oc,bck->bok', w_qkv, xb)       # [B_LOC, 128, K]

    # BN1: exact global stats via cross-core psum
    s1 = lax.psum(qkv.sum((0, 2)), 'i')
    s2 = lax.psum(jnp.square(qkv).sum((0, 2)), 'i')
    m = s1 / N_BN1
    v = s2 / N_BN1 - jnp.square(m)
    scale = bn_qkv_g / jnp.sqrt(v + EPS)
    qkv = qkv * scale[None, :, None] + (bn_qkv_b - m * scale)[None, :, None]

    qkv = qkv.reshape(B_LOC, G, GP * 2, K)
    q = qkv[:, :, :GP // 2]
    k = qkv[:, :, GP // 2:GP]
    vv = qkv[:, :, GP:]

    qr = jnp.einsum('bgci,cij->bgij', q, q_emb)
    kr = jnp.einsum('bgci,cij->bgij', k, k_emb).transpose(0, 1, 3, 2)
    qk = jnp.einsum('bgci,bgcj->bgij', q, k)

    ss = jnp.concatenate([qk, qr, kr], axis=1)       # [B_LOC, 3G, K, K]
    t1 = lax.psum(ss.sum((0, 2, 3)), 'i')
    t2 = lax.psum(jnp.square(ss).sum((0, 2, 3)), 'i')
    ms = t1 / N_BN2
    vs = t2 / N_BN2 - jnp.square(ms)
    ssc = bn_sim_g / jnp.sqrt(vs + EPS)
    ss = ss * ssc[None, :, None, None] + (bn_sim_b - ms * ssc)[None, :, None, None]

    sim = ss.reshape(B_LOC, 3, G, K, K).sum(axis=1)
    sim = jax.nn.softmax(sim, axis=3)

    sv = jnp.einsum('bgij,bgcj->bgci', sim, vv)
    sve = jnp.einsum('bgij,cij->bgci', sim, v_emb)
    so = jnp.concatenate([sv, sve], axis=-1).reshape(B_LOC, OP * 2, K)

    u1 = lax.psum(so.sum((0, 2)), 'i')
    u2 = lax.psum(jnp.square(so).sum((0, 2)), 'i')
    mo = u1 / N_BN1
    vo = u2 / N_BN1 - jnp.square(mo)
    osc = bn_out_g / jnp.sqrt(vo + EPS)
    so = so * osc[None, :, None] + (bn_out_b - mo * osc)[None, :, None]

    out = so.reshape(1, D1 // NCORES, D2, OP, 2, K).sum(axis=-2)
    return jnp.transpose(out, (0, 3, 1, 4, 2))       # [1, OP, d1l, K, D2]


_PMAPPED = jax.pmap(_shard_fn, axis_name='i',
                    in_axes=(0,) + (None,) * 10)


def kernel(x, w_qkv, bn_qkv_g, bn_qkv_b, bn_sim_g, bn_sim_b,
           bn_out_g, bn_out_b, relative, **_unused):
    x = np.asarray(x, np.float32)
    relative = np.asarray(relative, np.float32)

    # static relative-position gather done on host (index bookkeeping only)
    qi = np.arange(K)[None, :]
    ki = np.arange(K)[:, None]
    flat = (ki - qi + K - 1).reshape(-1)
    emb = relative[:, flat].reshape(GP * 2, K, K)
    q_emb = emb[:GP // 2]
    k_emb = emb[GP // 2:GP]
    v_emb = emb[GP:]

    # shard x along D1 (axis 2): [8, 1, C, D1/8, K, D2]
    xs = np.stack(np.split(x, NCORES, axis=2), axis=0)

    out_sh = _PMAPPED(jnp.asarray(xs), jnp.asarray(w_qkv),
                      jnp.asarray(bn_qkv_g), jnp.asarray(bn_qkv_b),
                      jnp.asarray(bn_sim_g), jnp.asarray(bn_sim_b),
                      jnp.asarray(bn_out_g), jnp.asarray(bn_out_b),
                      jnp.asarray(q_emb), jnp.asarray(k_emb), jnp.asarray(v_emb))
    out_sh = np.asarray(out_sh)                      # [8, 1, OP, d1l, K, D2]
    return np.concatenate(list(out_sh), axis=2).astype(np.float32)


# revision 3
# speedup vs baseline: 1.1411x; 1.1411x over previous
import numpy as np
import jax
import jax.numpy as jnp
from jax import lax

# Problem constants (hardcoded per spec: nn_AxialAttentionWithPosition3D)
G = 8        # groups
GP = 8       # group planes
K = 56       # attention axis length
OP = 64      # out planes
EPS = 1e-5
NCORES = 8
D1 = 32      # seq axis, sharded 4 per core
D2 = 32
C_IN = 64
B_LOC = (D1 // NCORES) * D2   # 128 positions per core
N_BN1 = NCORES * B_LOC * K    # global BN1/BN3 sample count per channel
N_BN2 = NCORES * B_LOC * K * K

jax.config.update("jax_default_matmul_precision", "highest")


def _shard_fn(xs, w_qkv, bn_qkv_g, bn_qkv_b, bn_sim_g, bn_sim_b,
              bn_out_g, bn_out_b, q_emb, k_emb, v_emb):
    # xs: [1, 64, D1/8, K, D2] slab of x along D1
    xp = jnp.transpose(xs, (0, 2, 4, 1, 3))          # [1, d1l, D2, C, K]
    xb = xp.reshape(B_LOC, C_IN, K)

    qkv = jnp.einsum('oc,bck->bok', w_qkv, xb)       # [B_LOC, 128, K]

    # BN1: exact global stats via one merged psum
    st = lax.psum(jnp.concatenate([qkv.sum((0, 2)),
                                   jnp.square(qkv).sum((0, 2))]), 'i')
    m = st[:128] / N_BN1
    v = st[128:] / N_BN1 - jnp.square(m)
    scale = bn_qkv_g / jnp.sqrt(v + EPS)
    qkv = qkv * scale[None, :, None] + (bn_qkv_b - m * scale)[None, :, None]

    qkv = qkv.reshape(B_LOC, G, GP * 2, K)
    q = qkv[:, :, :GP // 2]
    k = qkv[:, :, GP // 2:GP]
    vv = qkv[:, :, GP:]

    qr = jnp.einsum('bgci,cij->bgij', q, q_emb)
    kr = jnp.einsum('bgcj,cji->bgij', k, k_emb)      # pre-transposed form
    qk = jnp.einsum('bgci,bgcj->bgij', q, k)

    # BN2 stats per 24 channels without materializing concat(ss)
    sums = jnp.stack([qk.sum((0, 2, 3)), qr.sum((0, 2, 3)), kr.sum((0, 2, 3)),
                      jnp.square(qk).sum((0, 2, 3)), jnp.square(qr).sum((0, 2, 3)),
                      jnp.square(kr).sum((0, 2, 3))])          # [6, G]
    st2 = lax.psum(sums, 'i')
    ms = st2[:3] / N_BN2                                        # [3, G]
    vs = st2[3:] / N_BN2 - jnp.square(ms)
    g2 = bn_sim_g.reshape(3, G)
    b2 = bn_sim_b.reshape(3, G)
    a = g2 / jnp.sqrt(vs + EPS)                                 # [3, G]
    cst = (b2 - ms * a).sum(0)                                  # [G]
    sim = (a[0][None, :, None, None] * qk
           + a[1][None, :, None, None] * qr
           + a[2][None, :, None, None] * kr
           + cst[None, :, None, None])
    sim = jax.nn.softmax(sim, axis=3)

    sv = jnp.einsum('bgij,bgcj->bgci', sim, vv)      # [B, G, GP, K]
    sve = jnp.einsum('bgij,cij->bgci', sim, v_emb)

    # BN3 stats per 128 channels; channel map ch = g*16 + c*2 + h (h: 0=sv,1=sve)
    st3 = lax.psum(jnp.concatenate(
        [jnp.stack([sv.sum((0, 3)), sve.sum((0, 3))], axis=-1).reshape(-1),
         jnp.stack([jnp.square(sv).sum((0, 3)), jnp.square(sve).sum((0, 3))],
                   axis=-1).reshape(-1)]), 'i')
    mo = st3[:128].reshape(G, GP, 2) / N_BN1
    vo = st3[128:].reshape(G, GP, 2) / N_BN1 - jnp.square(mo)
    go = bn_out_g.reshape(G, GP, 2)
    bo = bn_out_b.reshape(G, GP, 2)
    osc = go / jnp.sqrt(vo + EPS)                    # [G, GP, 2]
    ocst = (bo - mo * osc).sum(-1)                   # [G, GP]
    out = (osc[None, :, :, 0, None] * sv
           + osc[None, :, :, 1, None] * sve
           + ocst[None, :, :, None])                 # [B, G, GP, K]

    out = out.reshape(1, D1 // NCORES, D2, OP, K)
    return jnp.transpose(out, (0, 3, 1, 4, 2))       # [1, OP, d1l, K, D2]


_PMAPPED = jax.pmap(_shard_fn, axis_name='i',
                    in_axes=(0,) + (None,) * 10)


def kernel(x, w_qkv, bn_qkv_g, bn_qkv_b, bn_sim_g, bn_sim_b,
           bn_out_g, bn_out_b, relative, **_unused):
    x = np.asarray(x, np.float32)
    relative = np.asarray(relative, np.float32)

    # static relative-position gather done on host (index bookkeeping only)
    qi = np.arange(K)[None, :]
    ki = np.arange(K)[:, None]
    flat = (ki - qi + K - 1).reshape(-1)
    emb = relative[:, flat].reshape(GP * 2, K, K)
    q_emb = emb[:GP // 2]
    k_emb = emb[GP // 2:GP]   # consumed via 'cji' subscript (pre-transposed kr)
    v_emb = emb[GP:]

    # shard x along D1 (axis 2): [8, 1, C, D1/8, K, D2]
    xs = np.stack(np.split(x, NCORES, axis=2), axis=0)

    out_sh = _PMAPPED(jnp.asarray(xs), jnp.asarray(w_qkv),
                      jnp.asarray(bn_qkv_g), jnp.asarray(bn_qkv_b),
                      jnp.asarray(bn_sim_g), jnp.asarray(bn_sim_b),
                      jnp.asarray(bn_out_g), jnp.asarray(bn_out_b),
                      jnp.asarray(q_emb), jnp.asarray(k_emb), jnp.asarray(v_emb))
    out_sh = np.asarray(out_sh)                      # [8, 1, OP, d1l, K, D2]
    return np.concatenate(list(out_sh), axis=2).astype(np.float32)


# revision 4
# speedup vs baseline: 1.4149x; 1.2399x over previous
import numpy as np
import jax
import jax.numpy as jnp
from jax import lax

# Problem constants (hardcoded per spec: nn_AxialAttentionWithPosition3D)
G = 8        # groups
GP = 8       # group planes
K = 56       # attention axis length
OP = 64      # out planes
EPS = 1e-5
NCORES = 8
D1 = 32      # seq axis, sharded 4 per core
D2 = 32
C_IN = 64
B_LOC = (D1 // NCORES) * D2   # 128 positions per core
N_BN1 = NCORES * B_LOC * K    # global BN1/BN3 sample count per channel
N_BN2 = NCORES * B_LOC * K * K

jax.config.update("jax_default_matmul_precision", "default")


def _shard_fn(xs, w_qkv, bn_qkv_g, bn_qkv_b, bn_sim_g, bn_sim_b,
              bn_out_g, bn_out_b, q_emb, k_emb, v_emb):
    # xs: [1, 64, D1/8, K, D2] slab of x along D1
    xp = jnp.transpose(xs, (0, 2, 4, 1, 3))          # [1, d1l, D2, C, K]
    xb = xp.reshape(B_LOC, C_IN, K)

    qkv = jnp.einsum('oc,bck->bok', w_qkv, xb)       # [B_LOC, 128, K]

    # BN1: exact global stats via one merged psum
    st = lax.psum(jnp.concatenate([qkv.sum((0, 2)),
                                   jnp.square(qkv).sum((0, 2))]), 'i')
    m = st[:128] / N_BN1
    v = st[128:] / N_BN1 - jnp.square(m)
    scale = bn_qkv_g / jnp.sqrt(v + EPS)
    qkv = qkv * scale[None, :, None] + (bn_qkv_b - m * scale)[None, :, None]

    qkv = qkv.reshape(B_LOC, G, GP * 2, K)
    q = qkv[:, :, :GP // 2]
    k = qkv[:, :, GP // 2:GP]
    vv = qkv[:, :, GP:]

    qr = jnp.einsum('bgci,cij->bgij', q, q_emb)
    kr = jnp.einsum('bgcj,cji->bgij', k, k_emb)      # pre-transposed form
    qk = jnp.einsum('bgci,bgcj->bgij', q, k)

    # BN2 stats per 24 channels without materializing concat(ss)
    sums = jnp.stack([qk.sum((0, 2, 3)), qr.sum((0, 2, 3)), kr.sum((0, 2, 3)),
                      jnp.square(qk).sum((0, 2, 3)), jnp.square(qr).sum((0, 2, 3)),
                      jnp.square(kr).sum((0, 2, 3))])          # [6, G]
    st2 = lax.psum(sums, 'i')
    ms = st2[:3] / N_BN2                                        # [3, G]
    vs = st2[3:] / N_BN2 - jnp.square(ms)
    g2 = bn_sim_g.reshape(3, G)
    b2 = bn_sim_b.reshape(3, G)
    a = g2 / jnp.sqrt(vs + EPS)                                 # [3, G]
    cst = (b2 - ms * a).sum(0)                                  # [G]
    sim = (a[0][None, :, None, None] * qk
           + a[1][None, :, None, None] * qr
           + a[2][None, :, None, None] * kr
           + cst[None, :, None, None])
    sim = jax.nn.softmax(sim, axis=3)

    sv = jnp.einsum('bgij,bgcj->bgci', sim, vv)      # [B, G, GP, K]
    sve = jnp.einsum('bgij,cij->bgci', sim, v_emb)

    # BN3 stats per 128 channels; channel map ch = g*16 + c*2 + h (h: 0=sv,1=sve)
    st3 = lax.psum(jnp.concatenate(
        [jnp.stack([sv.sum((0, 3)), sve.sum((0, 3))], axis=-1).reshape(-1),
         jnp.stack([jnp.square(sv).sum((0, 3)), jnp.square(sve).sum((0, 3))],
                   axis=-1).reshape(-1)]), 'i')
    mo = st3[:128].reshape(G, GP, 2) / N_BN1
    vo = st3[128:].reshape(G, GP, 2) / N_BN1 - jnp.square(mo)
    go = bn_out_g.reshape(G, GP, 2)
    bo = bn_out_b.reshape(G, GP, 2)
    osc = go / jnp.sqrt(vo + EPS)                    # [G, GP, 2]
    ocst = (bo - mo * osc).sum(-1)                   # [G, GP]
    out = (osc[None, :, :, 0, None] * sv
           + osc[None, :, :, 1, None] * sve
           + ocst[None, :, :, None])                 # [B, G, GP, K]

    out = out.reshape(1, D1 // NCORES, D2, OP, K)
    return jnp.transpose(out, (0, 3, 1, 4, 2))       # [1, OP, d1l, K, D2]


_PMAPPED = jax.pmap(_shard_fn, axis_name='i',
                    in_axes=(0,) + (None,) * 10)


def kernel(x, w_qkv, bn_qkv_g, bn_qkv_b, bn_sim_g, bn_sim_b,
           bn_out_g, bn_out_b, relative, **_unused):
    x = np.asarray(x, np.float32)
    relative = np.asarray(relative, np.float32)

    # static relative-position gather done on host (index bookkeeping only)
    qi = np.arange(K)[None, :]
    ki = np.arange(K)[:, None]
    flat = (ki - qi + K - 1).reshape(-1)
    emb = relative[:, flat].reshape(GP * 2, K, K)
    q_emb = emb[:GP // 2]
    k_emb = emb[GP // 2:GP]   # consumed via 'cji' subscript (pre-transposed kr)
    v_emb = emb[GP:]

    # shard x along D1 (axis 2): [8, 1, C, D1/8, K, D2]
    xs = np.stack(np.split(x, NCORES, axis=2), axis=0)

    out_sh = _PMAPPED(jnp.asarray(xs), jnp.asarray(w_qkv),
                      jnp.asarray(bn_qkv_g), jnp.asarray(bn_qkv_b),
                      jnp.asarray(bn_sim_g), jnp.asarray(bn_sim_b),
                      jnp.asarray(bn_out_g), jnp.asarray(bn_out_b),
                      jnp.asarray(q_emb), jnp.asarray(k_emb), jnp.asarray(v_emb))
    out_sh = np.asarray(out_sh)                      # [8, 1, OP, d1l, K, D2]
    return np.concatenate(list(out_sh), axis=2).astype(np.float32)
